# revision 1
# baseline (speedup 1.0000x reference)
"""Trainium2 Bass kernel for nn_CentroidEstimator (segment_reduce).

Full-input contract: kernel(**inputs) takes the complete arrays and returns
the complete (D+1, F, K) output. Internally:

  - Sharding: feature-parallel over F across 8 cores (64 columns each).
    Every core contracts over the full batch, so no cross-core collective
    is needed at all (the per-domain sums are computed whole on each core
    for its F-slice).
  - Host-side sharding prep: the batch is permuted so rows are grouped by
    domain and each domain is zero-padded to a multiple of 128. Every
    128-row contraction tile is then domain-pure, and the segmented
    reduction is expressed as per-domain PSUM accumulation groups - no
    one-hot mask materialization on device.
  - Transposed layout: lhsT = probs tile (128, K) so PSUM output is
    (K, 1+FL) with K on partitions: column 0 is the denominator (via a
    ones column streamed with the features), columns 1: are the numerator
    transposed. The divide becomes a per-partition tensor_scalar multiply.

B=4096, F=512, K=64, D=4 hardcoded from the problem spec.
"""

import numpy as np

ALPHA = 0.9
EPS = 1e-3
B, F, K, D = 4096, 512, 64, 4
NCORES = 8
FL = F // NCORES  # 64 feature columns per core
P = 128  # contraction tile rows (SBUF partitions)


# ---------------------------------------------------------------------------
# Host-side sharding prep
# ---------------------------------------------------------------------------

def _plan_tiles(dom: np.ndarray):
    """Group batch rows by domain, pad each domain to a multiple of P.

    Returns (idx, dom_of_tile, T): idx is (T*P,) row indices into the
    original batch with B as the sentinel for zero-pad rows; dom_of_tile
    maps each contraction tile to its (single) domain.
    """
    order = np.argsort(dom, kind="stable")
    counts = np.bincount(dom, minlength=D)
    tiles_d = np.maximum(1, -(-counts // P))  # ceil, at least one tile
    T = int(tiles_d.sum())
    idx = np.full((T * P,), B, dtype=np.int64)
    pos = 0
    off = 0
    for d in range(D):
        n = int(counts[d])
        idx[pos:pos + n] = order[off:off + n]
        off += n
        pos += int(tiles_d[d]) * P
    dom_of_tile = np.repeat(np.arange(D), tiles_d)
    return idx, dom_of_tile, T


def _pack_inputs(features, domains, cluster_probabilities, global_state,
                 domain_states):
    """Build per-core in_maps (and the tile->domain plan)."""
    dom = np.asarray(domains).reshape(-1).astype(np.int64)
    feats = np.asarray(features, dtype=np.float32)
    probs = np.asarray(cluster_probabilities, dtype=np.float32)
    gstate = np.asarray(global_state, dtype=np.float32)
    dstates = np.asarray(domain_states, dtype=np.float32)

    idx, dom_of_tile, T = _plan_tiles(dom)

    import ml_dtypes
    bf16 = ml_dtypes.bfloat16

    # Gather once with a zero sentinel row appended (pad rows -> zeros).
    feats_x = np.concatenate([feats, np.zeros((1, F), np.float32)], axis=0)[idx]
    probs_x = np.concatenate([probs, np.zeros((1, K), np.float32)], axis=0)[idx]

    # probsp: (P, T, K), partition-major so each SBUF partition's bytes are
    # one contiguous run in DRAM. Shared by all cores. bf16: the matmul
    # accumulates fp32 in PSUM; operand rounding keeps rel err ~3e-3.
    probsp = np.ascontiguousarray(
        probs_x.reshape(T, P, K).transpose(1, 0, 2)).astype(bf16)

    in_maps = []
    for c in range(NCORES):
        sl = slice(FL * c, FL * (c + 1))
        fa = np.empty((T * P, FL + 1), np.float32)
        fa[:, 0] = 1.0  # ones column -> denominator row of the matmul
        fa[:, 1:] = feats_x[:, sl]
        featp = np.ascontiguousarray(
            fa.reshape(T, P, FL + 1).transpose(1, 0, 2)).astype(bf16)
        st_dT = np.ascontiguousarray(dstates[:, sl, :].transpose(2, 0, 1))
        st_gT = np.ascontiguousarray(gstate[sl, :].T)
        in_maps.append({
            "featp": featp,
            "probsp": probsp,
            "st_dT": st_dT,
            "st_gT": st_gT,
        })
    return in_maps, dom_of_tile, T


# ---------------------------------------------------------------------------
# Bass program
# ---------------------------------------------------------------------------

def build_nc(T, dom_of_tile):
    import concourse.bacc as bacc
    import concourse.tile as tile
    from concourse import mybir

    dt = mybir.dt.float32
    bf = mybir.dt.bfloat16
    nc = bacc.Bacc("TRN2", target_bir_lowering=False)

    featp_d = nc.dram_tensor("featp", [P, T, FL + 1], bf, kind="ExternalInput")
    probsp_d = nc.dram_tensor("probsp", [P, T, K], bf, kind="ExternalInput")
    stdT_d = nc.dram_tensor("st_dT", [K, D, FL], dt, kind="ExternalInput")
    stgT_d = nc.dram_tensor("st_gT", [K, FL], dt, kind="ExternalInput")
    outT_d = nc.dram_tensor("outT", [K, D + 1, FL], bf, kind="ExternalOutput")

    add = mybir.AluOpType.add
    mult = mybir.AluOpType.mult
    W = FL + 1  # per-domain psum column block: [den | num_f...]

    with tile.TileContext(nc) as tc:
        with (
            tc.tile_pool(name="io", bufs=1) as io,
            tc.tile_pool(name="ps", bufs=1, space="PSUM") as ps,
        ):
            featp = io.tile([P, T, FL + 1], bf)
            probsp = io.tile([P, T, K], bf)
            # Graduated chunks, one tensor per HWDGE ring (the two rings
            # share a descriptor-rate-bound ~250 GB/s aggregate; small
            # first chunks let the PE start early). The SWDGE (gpsimd)
            # ring is ~3x slower - states only.
            fb = sorted({0, (15 * T) // 100, (40 * T) // 100,
                         (70 * T) // 100, T})
            for a, b in zip(fb[:-1], fb[1:]):
                nc.sync.dma_start(
                    out=featp[:, a:b, :], in_=featp_d[:, a:b, :])
            for a, b in zip(fb[:-1], fb[1:]):
                nc.scalar.dma_start(
                    out=probsp[:, a:b, :], in_=probsp_d[:, a:b, :])
            stdT = io.tile([K, D, FL], dt)
            stgT = io.tile([K, FL], dt)
            nc.gpsimd.dma_start(out=stdT[:], in_=stdT_d[:])
            nc.gpsimd.dma_start(out=stgT[:], in_=stgT_d[:])
            # Prescale states by ALPHA early (overlaps the input DMAs) so
            # each section's EMA is a single scalar_tensor_tensor later.
            std_s = io.tile([K, D, FL], dt)
            stg_s = io.tile([K, FL], dt)
            nc.vector.tensor_scalar_mul(std_s[:], stdT[:], ALPHA)
            nc.vector.tensor_scalar_mul(stg_s[:], stgT[:], ALPHA)

            # One PSUM bank per domain so the DVE's per-domain tail reads
            # of bank d overlap the PE's writes into bank d+1 (same-bank
            # PE-write/DVE-read would be serialized by Tile).
            psums = [ps.tile([K, W], dt, name=f"psum{d}") for d in range(D)]
            outT = io.tile([K, D + 1, FL], bf)
            rec = io.tile([K, D + 1], dt)
            denc = io.tile([K, D + 1], dt)
            ng = io.tile([K, W], dt)
            for d in range(D):
                ts_d = [t for t in range(T) if dom_of_tile[t] == d]
                last = len(ts_d) - 1
                for j, t in enumerate(ts_d):
                    nc.tensor.matmul(
                        psums[d][:],
                        probsp[:, t, :],   # lhsT (stationary): (128, K)
                        featp[:, t, :],    # rhs (moving): (128, 1+FL)
                        start=(j == 0),
                        stop=(j == last),
                    )
                # Per-domain tail under the next domain's matmuls; the ng
                # accumulation goes first so the global chain (which is the
                # last consumer) unblocks as early as possible.
                if d == 0:
                    nc.vector.tensor_copy(ng[:], psums[0][:])
                else:
                    nc.vector.tensor_add(ng[:], ng[:], psums[d][:])
                if d == D - 1:
                    # The global section's writeback is the kernel's last
                    # byte - run its chain before the last domain's.
                    with tc.high_priority():
                        nc.vector.tensor_scalar(
                            denc[:, D:D + 1], ng[:, 0:1],
                            EPS, 1.0 / (1.0 - ALPHA), op0=add, op1=mult)
                        nc.vector.reciprocal(rec[:, D:D + 1],
                                             denc[:, D:D + 1])
                        nc.vector.scalar_tensor_tensor(
                            out=outT[:, 0, :],
                            in0=ng[:, 1:], scalar=rec[:, D:D + 1],
                            in1=stg_s[:], op0=mult, op1=add)
                        nc.scalar.dma_start(out=outT_d[:, 0, :],
                                            in_=outT[:, 0, :])
                nc.vector.tensor_scalar(
                    denc[:, d:d + 1], psums[d][:, 0:1],
                    EPS, 1.0 / (1.0 - ALPHA), op0=add, op1=mult)
                nc.vector.reciprocal(rec[:, d:d + 1], denc[:, d:d + 1])
                nc.vector.scalar_tensor_tensor(
                    out=outT[:, 1 + d, :],
                    in0=psums[d][:, 1:], scalar=rec[:, d:d + 1],
                    in1=std_s[:, d, :], op0=mult, op1=add)
                if d == 1:
                    # Domains 0-1 are final; start their writeback early on
                    # the idle sync ring.
                    nc.sync.dma_start(
                        out=outT_d[:, 1:3, :], in_=outT[:, 1:3, :])
            nc.sync.dma_start(out=outT_d[:, 3:, :], in_=outT[:, 3:, :])

    _strip_const_preamble(nc, mybir)
    nc.compile()
    return nc


def _strip_const_preamble(nc, mybir):
    """Remove the framework's const-AP memsets (and the drain they force)
    from the preamble. Safe only because this kernel never reads the
    const-* tensors - asserted below."""
    def _names(args):
        for a in args:
            t = getattr(getattr(a, "bass_ap", None), "tensor", None)
            nm = getattr(t, "name", "") or ""
            if nm.startswith("const-"):
                yield nm
    for bb in nc.main_func.blocks:
        keep = []
        for ins in bb.instructions:
            if isinstance(ins, mybir.InstMemset) and any(_names(ins.outs)):
                continue
            assert not any(_names(ins.ins)), (
                f"{ins.name} reads a const-AP tensor; cannot strip preamble")
            keep.append(ins)
        bb.instructions[:] = keep


# ---------------------------------------------------------------------------
# Entry point
# ---------------------------------------------------------------------------

def _assemble(results):
    out = np.empty((D + 1, F, K), np.float32)
    for c in range(NCORES):
        res = results[c]["outT"]  # (K, D+1, FL)
        out[:, FL * c:FL * (c + 1), :] = res.transpose(1, 2, 0)
    return out


def kernel(features, domains, cluster_probabilities, global_state,
           domain_states, _trace=False):
    from concourse.bass_utils import run_bass_kernel_spmd

    in_maps, dom_of_tile, T = _pack_inputs(
        features, domains, cluster_probabilities, global_state, domain_states)
    nc = build_nc(T, dom_of_tile)
    res = run_bass_kernel_spmd(
        nc, in_maps, core_ids=list(range(NCORES)), trace=_trace)
    out = _assemble(res.results)
    if _trace:
        kernel.last_exec_time_ns = res.exec_time_ns
        kernel.last_results = res
    return out


if __name__ == "__main__":
    # Smoke test with random data (no reference available standalone).
    rng = np.random.default_rng(0)
    inputs = {
        "features": rng.standard_normal((B, F)).astype(np.float32),
        "domains": rng.integers(0, D, (1, B)).astype(np.int64),
        "cluster_probabilities": rng.random((B, K)).astype(np.float32),
        "global_state": np.zeros((F, K), np.float32),
        "domain_states": np.zeros((D, F, K), np.float32),
    }
    out = kernel(**inputs)
    print("out", out.shape, out.dtype, float(np.abs(out).max()))

